# revision 38
# baseline (speedup 1.0000x reference)
"""Multi-head self-attention (RoPE, causal) Trainium2 Bass kernel, 8 NeuronCores.

Sharding: data-parallel over batch (B=2) x tensor-parallel over heads
(16 heads -> 4 groups of 4). Core c handles batch b=c//4, heads 4*(c%4)..4*(c%4)+3.
Each core computes its 4 heads' attention plus a partial output projection;
the host sums the 4 bf16 partial outputs per batch element in f32.

Single software-pipelined phase per core (vs the old serial phases):
QKV projection of chunk c+1, attention for chunk c, and output
projection (per l-tile, gated on its q-chunk's drain) are EMITTED
interleaved at matmul granularity with fractional pacing, so the
in-order tensor-engine queue always has filler work during the
activation engine's softmax-exp latency (exp is the co-bottleneck:
~80us at 128 lanes/cycle incl. overheads, vs ~100us of PE streaming).

Layouts / key tricks (per core):
  x^T [1024d, L] bf16 as one [128, 8, L] tile, DMA'd in 512-col chunks,
  pipelined 2 chunks ahead; 8 dummy warm-up matmuls ramp the PE p-state
  during the initial DMA window.
  Q^T/K^T [256c, L] = W_slice @ x^T; RoPE in [channel, L] layout via
  DVE stream_shuffle pair-swap + cos/signed-sin (shuffle+mul DVE,
  mul+add GpSimd; GpSimd cannot read PSUM).
  Scores T = K^T.T @ Q^T per head in [k, q] 512-wide q-chunks; the two
  heads' QK matmuls stream CONCURRENTLY via tile_position (0,0)/(64,0).
  Causal masking costs no vector work: a tiny extra matmul accumulates
  -30000 above the diagonal in PSUM (lhsT = strict-upper-tri const,
  rhs = doubled identity covering both heads), so exp -> 0.
  exp per k-tile covers both heads [128, <=1024] on the scalar engine.
  PV: P^T @ [V_h | ones*64] stationary M=128 -- the ones block
  replicated 64x makes po[64:128] hold the softmax denominators
  pre-broadcast across 64 partitions: drain = shifted tensor_copy down
  (standard copy honors shifted base partitions; custom-DVE ops do
  not), reciprocal_approx_fast, two muls; no DRAM round-trip.
  PSUM discipline: first writer of each 2KB bank uses start=True
  (marks the whole zero-region), later disjoint writers start=False
  accumulate onto pending-zero.  8 banks: scores 2x2, po 2, proj 2.
  Output projection per l-tile from at [256c, L] x Wo^T, staged bf16
  on DVE, bf16 partials summed on host in f32.
"""
import sys, math

sys.path.insert(0, "/opt/trn_rl_repo")

import numpy as np
import ml_dtypes

import concourse.bacc as bacc
import concourse.bass as bass
import concourse.mybir as mybir
import concourse.tile as tile
from concourse.bass_utils import run_bass_kernel_spmd

BF16 = mybir.dt.bfloat16
F32 = mybir.dt.float32
NPBF16 = ml_dtypes.bfloat16

D_MODEL = 1024
D_HEAD = 64
HALF = D_HEAD // 2
ROPE_THETA = 10000.0
N_CORES = 8
C = 256            # channels per core (4 heads x 64)
PCW = 512          # projection chunk width (L columns)
QW = 256           # attention q-chunk width
SWAP32 = [i ^ 1 for i in range(32)]
SCALE = 1.0 / math.sqrt(D_HEAD)


def _body(nc, tc, L, pp, rpp, sbp, drp, osp, scp, pop, pjp):
    n_pc = L // PCW          # projection chunks
    n_qc = L // QW           # attention q chunks
    n_lt = L // 128          # l-tiles / k-tiles

    xt_d = nc.dram_tensor("xt", [D_MODEL, L], BF16, kind="ExternalInput").ap()
    wq_d = nc.dram_tensor("wqt", [D_MODEL, C], BF16, kind="ExternalInput").ap()
    wk_d = nc.dram_tensor("wkt", [D_MODEL, C], BF16, kind="ExternalInput").ap()
    wv_d = nc.dram_tensor("wvt", [D_MODEL, C], BF16, kind="ExternalInput").ap()
    wo_d = nc.dram_tensor("wot", [C, D_MODEL], BF16, kind="ExternalInput").ap()
    cos_d = nc.dram_tensor("cosb", [128, L], BF16, kind="ExternalInput").ap()
    sin_d = nc.dram_tensor("ssin", [128, L], BF16, kind="ExternalInput").ap()
    mk_d = nc.dram_tensor("masks", [128, 128], BF16, kind="ExternalInput").ap()
    eye_d = nc.dram_tensor("eye", [128, 256], BF16, kind="ExternalInput").ap()
    out_d = nc.dram_tensor("out", [L, D_MODEL], BF16, kind="ExternalOutput").ap()

    # ---- persistent SBUF tensors
    wq = pp.tile([128, 8, C], BF16)
    wk = pp.tile([128, 8, C], BF16)
    wv = pp.tile([128, 8, C], BF16)
    wo = pp.tile([128, 2, D_MODEL], BF16)
    cs = pp.tile([128, L], BF16)
    sn = pp.tile([128, L], BF16)
    mtriT = pp.tile([128, 128], BF16)   # strict upper-tri -30000 (mask bias^T)
    eye = pp.tile([128, 256], BF16)     # identity, doubled
    xb = pp.tile([128, 8, L], BF16)     # x^T, d-tile-major
    qtb = pp.tile([128, 2, L], BF16)
    ktb = pp.tile([128, 2, L], BF16)
    # V per k-tile: head hd at [128*hd, 128*hd+64), ones at [128*hd+64, ...)
    vt = pp.tile([128, n_lt, 512], BF16)
    at = pp.tile([128, 2, L], BF16)

    # ---- input DMA: everything on the sync queue (scalar stays clear for exp);
    # later chunks' x loads are emitted inside the main loop so store DMAs
    # interleave rather than queueing behind them.
    xt_t = xt_d.rearrange("(a p) l -> p a l", p=128)
    nc.sync.dma_start(out=wq[:], in_=wq_d.rearrange("(a p) c -> p a c", p=128))
    nc.sync.dma_start(out=xb[:, 0:2, 0:PCW], in_=xt_t[:, 0:2, 0:PCW])
    nc.sync.dma_start(out=xb[:, 2:4, 0:PCW], in_=xt_t[:, 2:4, 0:PCW])
    nc.sync.dma_start(out=xb[:, 4:6, 0:PCW], in_=xt_t[:, 4:6, 0:PCW])
    nc.sync.dma_start(out=xb[:, 6:8, 0:PCW], in_=xt_t[:, 6:8, 0:PCW])
    nc.sync.dma_start(out=wk[:], in_=wk_d.rearrange("(a p) c -> p a c", p=128))
    nc.sync.dma_start(out=cs[:], in_=cos_d)
    nc.sync.dma_start(out=sn[:], in_=sin_d)
    nc.sync.dma_start(out=mtriT[:], in_=mk_d)
    nc.sync.dma_start(out=eye[:], in_=eye_d)
    nc.sync.dma_start(out=wv[:], in_=wv_d.rearrange("(a p) c -> p a c", p=128))
    nc.sync.dma_start(out=xb[:, :, PCW:2 * PCW], in_=xt_t[:, :, PCW:2 * PCW])
    nc.sync.dma_start(out=wo[:], in_=wo_d.rearrange("(a p) e -> p a e", p=128))
    # warm the PE p-state during the initial DMA window: dummy matmuls on
    # a zeroed sbuf tile (results discarded; psum reset by the first real
    # start=True groups).  wrm memset first so warmups start immediately.
    wrm = pp.tile([128, 256], BF16, name="wrm")
    nc.gpsimd.memset(wrm[:], 0.0)
    for wi in range(8):
        wps = pjp.tile([128, PCW], F32, tag="pj", name=f"wps{wi}")
        nc.tensor.matmul(wps[:, 0:C], lhsT=wrm[:, 0:128], rhs=wrm[:, :],
                         start=True, stop=True, skip_group_check=True)
    # ones columns of vt (once)
    vtv = vt[:, :, :].rearrange("p k (hd x) -> p k hd x", x=128)
    nc.gpsimd.memset(vtv[:, :, :, 64:128], 1.0)

    # ---------------- streams ----------------
    def proj_qk_stream(c):
        ls = c * PCW
        if c + 2 < n_pc:  # prefetch chunk c+2 (c+1 already in flight)
            ns_ = (c + 2) * PCW
            nc.sync.dma_start(out=xb[:, :, ns_:ns_ + PCW],
                              in_=xt_t[:, :, ns_:ns_ + PCW])
        for nm, wt, dstb in (("q", wq, qtb), ("k", wk, ktb)):
            for ct in (0, 1):
                pj = pjp.tile([128, PCW], F32, tag="pj", name=f"pj_{nm}{ct}_{c}")
                for dt_ in range(8):
                    nc.tensor.matmul(
                        pj[:],
                        lhsT=wt[:, dt_, ct * 128:ct * 128 + 128],
                        rhs=xb[:, dt_, ls:ls + PCW],
                        start=(dt_ == 0), stop=(dt_ == 7))
                    if dt_ == 3:
                        yield
                # RoPE: r = pj*cos + shuffle(pj)*ssin
                sh = rpp.tile([128, PCW], F32, tag="sh", name=f"sh{nm}{ct}{c}")
                t1 = rpp.tile([128, PCW], BF16, tag="t1", name=f"t1{nm}{ct}{c}")
                t2 = rpp.tile([128, PCW], BF16, tag="t2", name=f"t2{nm}{ct}{c}")
                nc.vector.stream_shuffle(sh[:], pj[:], SWAP32)
                nc.vector.tensor_mul(t1[:], pj[:], cs[:, ls:ls + PCW])
                nc.gpsimd.tensor_mul(t2[:], sh[:], sn[:, ls:ls + PCW])
                nc.gpsimd.tensor_add(dstb[:, ct, ls:ls + PCW], t1[:], t2[:])
                yield
    def proj_v_stream(c):
        for lt in range(c * 4, c * 4 + 4):
            pj = pjp.tile([128, PCW], F32, tag="pj", name=f"pjv_{lt}")
            for dt_ in range(8):
                nc.tensor.matmul(
                    pj[:, 0:C],
                    lhsT=xb[:, dt_, lt * 128:lt * 128 + 128],
                    rhs=wv[:, dt_, :],
                    start=(dt_ == 0), stop=(dt_ == 7))
                if dt_ == 3:
                    yield
            nc.vector.tensor_copy(
                vtv[:, lt, :, 0:64],
                pj[:, 0:C].rearrange("p (hd x) -> p hd x", x=64))
            yield

    def proj_stream(c):
        yield from proj_qk_stream(c)
        yield from proj_v_stream(c)

    def attn_stream(qcs):
        for qc in qcs:
            for pair in (0, 1):
                qs = qc * PCW
                nkt = 4 * (qc + 1)
                po = pop.tile([128, 2, PCW], F32, tag="po", name=f"po_{qc}_{pair}")
                tiles = {}

                def qk(kt):
                    pt_ps = scp.tile([128, 2, PCW], F32, tag="sc",
                                     name=f"ps_{qc}_{pair}_{kt}")
                    pt_sb = sbp.tile([128, 2, PCW], BF16, tag="pt",
                                     name=f"pb_{qc}_{pair}_{kt}")
                    qlo = max(0, kt * 128 - qs)
                    diag = kt * 128 >= qs
                    for h in (0, 1):
                        nc.tensor.matmul(
                            pt_ps[:, h, qlo:PCW],
                            lhsT=ktb[64 * h:64 * h + 64, pair,
                                     kt * 128:kt * 128 + 128],
                            rhs=qtb[64 * h:64 * h + 64, pair,
                                    qs + qlo:qs + PCW],
                            start=True, stop=not diag,
                            tile_position=(64 * h, 0),
                            skip_group_check=True)
                    if diag:
                        boff = kt * 128 - qs
                        nc.tensor.matmul(
                            pt_ps[:, :, boff:boff + 128],
                            lhsT=mtriT[:, 0:128],
                            rhs=eye[:, :],
                            start=False, stop=True,
                            skip_group_check=True)
                    tiles[kt] = (pt_ps, pt_sb, qlo)

                def exp_pv(kt):
                    pt_ps, pt_sb, qlo = tiles.pop(kt)
                    nc.scalar.activation(
                        pt_sb[:, :, qlo:PCW], pt_ps[:, :, qlo:PCW],
                        mybir.ActivationFunctionType.Exp, scale=SCALE)
                    for h in (0, 1):
                        hd = 2 * pair + h
                        nc.tensor.matmul(
                            po[:, h, qlo:PCW],
                            lhsT=vt[:, kt, 128 * hd:128 * hd + 128],
                            rhs=pt_sb[:, h, qlo:PCW],
                            start=(kt == 0),
                            stop=(kt == nkt - 1),
                            skip_group_check=True)

                for kt in range(nkt):
                    qk(kt)
                    yield
                    if kt > 0:
                        exp_pv(kt - 1)
                        yield
                exp_pv(nkt - 1)
                # drain: denominators live in po[64:128] (ones-replicated).
                # Standard tensor_copy honors a shifted input base partition;
                # custom-DVE ops (reciprocal) do not -- copy down first.
                dcp = drp.tile([64, 2, PCW], F32, tag="dcp", name=f"dc_{qc}_{pair}")
                nc.vector.tensor_copy(dcp[:], po[64:128, :, :])
                pbi = drp.tile([64, 2, PCW], F32, tag="pbi", name=f"pi_{qc}_{pair}")
                nc.vector.reciprocal_approx_fast(out=pbi[:], in_=dcp[:])
                nc.vector.tensor_mul(at[0:64, pair, qs:qs + PCW],
                                     po[0:64, 0, :], pbi[:, 0, :])
                tmh = drp.tile([64, PCW], BF16, tag="tmh", name=f"th_{qc}_{pair}")
                nc.vector.tensor_mul(tmh[:], po[0:64, 1, :], pbi[:, 1, :])
                # final chain: Act's hwdge queue is idle by now, while sync
                # still drains out-stores -- route around the backlog
                deng = nc.scalar if (qc == n_pc - 1 and pair == 1) else nc.sync
                deng.dma_start(out=at[64:128, pair, qs:qs + PCW], in_=tmh[:])
                yield

    def oproj_stream(lts, store_eng=None):
        seng = store_eng
        for lt in lts:
            for eh in (0, 1):
                pj = pjp.tile([128, PCW], F32, tag="pj", name=f"pjo_{lt}_{eh}")
                nc.tensor.matmul(pj[:], lhsT=at[:, 0, lt * 128:lt * 128 + 128],
                                 rhs=wo[:, 0, eh * 512:eh * 512 + 512],
                                 start=True, stop=False, skip_group_check=True)
                yield
                nc.tensor.matmul(pj[:], lhsT=at[:, 1, lt * 128:lt * 128 + 128],
                                 rhs=wo[:, 1, eh * 512:eh * 512 + 512],
                                 start=False, stop=True, skip_group_check=True)
                yield
                stg = osp.tile([128, PCW], BF16, tag="stg", name=f"stg_{lt}_{eh}")
                nc.vector.tensor_copy(stg[:], pj[:])
                (seng or nc.sync).dma_start(
                    out=out_d[lt * 128:lt * 128 + 128,
                              eh * 512:eh * 512 + 512],
                    in_=stg[:])
                yield

    def run_all(gen):
        for _ in gen:
            pass

    def interleave(a, b, na, nb):
        """Pace nb filler units (b) evenly across na units of a."""
        ita, itb = iter(a), iter(b)
        alive_a = alive_b = True
        acc = 0.0
        step = nb / max(1, na)
        while alive_a or alive_b:
            if alive_a:
                try:
                    next(ita)
                except StopIteration:
                    alive_a = False
            acc += step if alive_a else 1e9
            while alive_b and acc >= 1.0:
                acc -= 1.0
                try:
                    next(itb)
                except StopIteration:
                    alive_b = False

    def chain(*gens):
        for g in gens:
            yield from g

    def n_attn(qcs):
        return sum(2 * (2 * 4 * (qc + 1) + 1) for qc in qcs)

    N_PROJ = 16  # yields per proj_stream chunk
    N_OP = 6     # yields per oproj l-tile

    run_all(proj_stream(0))
    interleave(attn_stream((0,)), proj_stream(1), n_attn((0,)), N_PROJ)
    interleave(attn_stream((1,)),
               chain(proj_stream(2), oproj_stream((0, 1, 2))),
               n_attn((1,)), N_PROJ + 3 * N_OP)
    interleave(attn_stream((2,)),
               chain(proj_qk_stream(3), oproj_stream((3, 4, 5))),
               n_attn((2,)), 8 + 3 * N_OP)
    interleave(attn_stream((3,)),
               chain(proj_v_stream(3), oproj_stream((6, 7, 8, 9, 10, 11))),
               n_attn((3,)), 8 + 6 * N_OP)
    run_all(oproj_stream((12, 13, 14, 15), store_eng=nc.scalar))


def build_nc(L=2048):
    """Build + compile the per-core Bass program (same NEFF on all 8 cores)."""
    assert L % PCW == 0
    nc = bacc.Bacc("TRN2", target_bir_lowering=False, debug=False,
                   num_devices=N_CORES)
    with tile.TileContext(nc) as tc:
        with tc.tile_pool(name="persist", bufs=1) as pp, \
             tc.tile_pool(name="ropet", bufs=2) as rpp, \
             tc.tile_pool(name="ptsb", bufs=4) as sbp, \
             tc.tile_pool(name="drain", bufs=4) as drp, \
             tc.tile_pool(name="ostg", bufs=3) as osp, \
             tc.tile_pool(name="sc_ps", bufs=2, space="PSUM") as scp, \
             tc.tile_pool(name="po_ps", bufs=1, space="PSUM") as pop, \
             tc.tile_pool(name="pj_ps", bufs=2, space="PSUM") as pjp:
            _body(nc, tc, L, pp, rpp, sbp, drp, osp, scp, pop, pjp)
    nc.compile()
    return nc


_NC_CACHE = {}


def _get_nc(L):
    if L not in _NC_CACHE:
        _NC_CACHE[L] = build_nc(L)
    return _NC_CACHE[L]


def make_inputs(x, token_positions, Wq, Wk, Wv, Wo):
    """Host-side shard/layout prep -> list of 8 per-core input dicts."""
    B, L, _ = x.shape
    pos = np.asarray(token_positions).astype(np.float64)
    S = ROPE_THETA ** (-2.0 / D_HEAD)
    thetas = S ** np.arange(HALF, dtype=np.float64)
    ang = pos[:, None] * thetas[None, :]          # [L, 32]
    cosL = np.cos(ang).T                          # [32, L]
    sinL = np.sin(ang).T
    # per-channel tables on the natural (head, dim) layout:
    # row p (within a 64-row head block): pair i = (p%64)//2
    # cosb[p] = cos(theta_i * pos); ssin[p] = -sin if dim even else +sin
    cosb = np.empty((128, L), dtype=np.float64)
    ssin = np.empty((128, L), dtype=np.float64)
    for p in range(128):
        i = (p % 64) // 2
        cosb[p] = cosL[i]
        ssin[p] = -sinL[i] if (p % 2 == 0) else sinL[i]
    cosb = cosb.astype(NPBF16)
    ssin = ssin.astype(NPBF16)

    r = np.arange(128)[:, None]
    col = np.arange(128)[None, :]
    # masks = mtriT: strict upper-triangular -30000.  Used as matmul lhsT
    # with rhs=I to add -30000 above the diagonal of score blocks (so the
    # device adds bias[p, j] = mtriT[j, p]... lhsT[d, p] applied as
    # (lhsT.T @ I)[p, j] = mtriT[j, p]; want -30000 where j < p.
    masks = np.where(r < col, -30000.0, 0.0).astype(NPBF16)
    eye = np.concatenate([np.eye(128), np.eye(128)], axis=1).astype(NPBF16)

    xts = [np.ascontiguousarray(x[b].astype(NPBF16).T) for b in range(B)]
    in_maps = []
    shard_cache = {}
    for core in range(N_CORES):
        b, hg = core // 4, core % 4
        if hg not in shard_cache:
            rows = slice(hg * 256, hg * 256 + 256)
            shard_cache[hg] = {
                "wqt": np.ascontiguousarray(Wq[rows].astype(NPBF16).T),
                "wkt": np.ascontiguousarray(Wk[rows].astype(NPBF16).T),
                "wvt": np.ascontiguousarray(Wv[rows].astype(NPBF16).T),
                "wot": np.ascontiguousarray(Wo[:, rows].astype(NPBF16).T),
            }
        m = dict(shard_cache[hg])
        m["xt"] = xts[b]
        m["cosb"] = cosb
        m["ssin"] = ssin
        m["masks"] = masks
        m["eye"] = eye
        in_maps.append(m)
    return in_maps


def kernel(x, token_positions, Wq, Wk, Wv, Wo):
    x = np.asarray(x); Wq = np.asarray(Wq); Wk = np.asarray(Wk)
    Wv = np.asarray(Wv); Wo = np.asarray(Wo)
    B, L, _ = x.shape
    nc = _get_nc(L)
    in_maps = make_inputs(x, token_positions, Wq, Wk, Wv, Wo)
    res = run_bass_kernel_spmd(nc, in_maps, core_ids=list(range(N_CORES)))
    out = np.zeros((B, L, D_MODEL), dtype=np.float32)
    for core in range(N_CORES):
        out[core // 4] += res.results[core]["out"].astype(np.float32)
    return out


# revision 40
# speedup vs baseline: 1.0066x; 1.0066x over previous
"""Multi-head self-attention (RoPE, causal) Trainium2 Bass kernel, 8 NeuronCores.

Sharding: data-parallel over batch (B=2) x tensor-parallel over heads
(16 heads -> 4 groups of 4). Core c handles batch b=c//4, heads 4*(c%4)..4*(c%4)+3.
Each core computes its 4 heads' attention plus a partial output projection;
the host sums the 4 bf16 partial outputs per batch element in f32.

Single software-pipelined phase per core (vs the old serial phases):
QKV projection of chunk c+1, attention for chunk c, and output
projection (per l-tile, gated on its q-chunk's drain) are EMITTED
interleaved at matmul granularity with fractional pacing, so the
in-order tensor-engine queue always has filler work during the
activation engine's softmax-exp latency (exp is the co-bottleneck:
~80us at 128 lanes/cycle incl. overheads, vs ~100us of PE streaming).

Layouts / key tricks (per core):
  x^T [1024d, L] bf16 as one [128, 8, L] tile, DMA'd in 512-col chunks,
  pipelined 2 chunks ahead; 8 dummy warm-up matmuls ramp the PE p-state
  during the initial DMA window.
  Q^T/K^T [256c, L] = W_slice @ x^T; RoPE in [channel, L] layout via
  DVE stream_shuffle pair-swap + cos/signed-sin (shuffle+mul DVE,
  mul+add GpSimd; GpSimd cannot read PSUM).
  Scores T = K^T.T @ Q^T per head in [k, q] 512-wide q-chunks; the two
  heads' QK matmuls stream CONCURRENTLY via tile_position (0,0)/(64,0).
  Causal masking costs no vector work: a tiny extra matmul accumulates
  -30000 above the diagonal in PSUM (lhsT = strict-upper-tri const,
  rhs = doubled identity covering both heads), so exp -> 0.
  exp per k-tile covers both heads [128, <=1024] on the scalar engine.
  PV: P^T @ [V_h | ones*64] stationary M=128 -- the ones block
  replicated 64x makes po[64:128] hold the softmax denominators
  pre-broadcast across 64 partitions: drain = shifted tensor_copy down
  (standard copy honors shifted base partitions; custom-DVE ops do
  not), reciprocal_approx_fast, two muls; no DRAM round-trip.
  PSUM discipline: first writer of each 2KB bank uses start=True
  (marks the whole zero-region), later disjoint writers start=False
  accumulate onto pending-zero.  8 banks: scores 2x2, po 2, proj 2.
  Output projection per l-tile from at [256c, L] x Wo^T, staged bf16
  on DVE, bf16 partials summed on host in f32.
"""
import sys, math

sys.path.insert(0, "/opt/trn_rl_repo")

import numpy as np
import ml_dtypes

import concourse.bacc as bacc
import concourse.bass as bass
import concourse.mybir as mybir
import concourse.tile as tile
from concourse.bass_utils import run_bass_kernel_spmd

BF16 = mybir.dt.bfloat16
F32 = mybir.dt.float32
NPBF16 = ml_dtypes.bfloat16

D_MODEL = 1024
D_HEAD = 64
HALF = D_HEAD // 2
ROPE_THETA = 10000.0
N_CORES = 8
C = 256            # channels per core (4 heads x 64)
PCW = 512          # projection chunk width (L columns)
QW = 256           # attention q-chunk width
SWAP32 = [i ^ 1 for i in range(32)]
SCALE = 1.0 / math.sqrt(D_HEAD)


def _body(nc, tc, L, pp, rpp, sbp, drp, osp, scp, pop, pjp):
    n_pc = L // PCW          # projection chunks
    n_qc = L // QW           # attention q chunks
    n_lt = L // 128          # l-tiles / k-tiles

    xt_d = nc.dram_tensor("xt", [D_MODEL, L], BF16, kind="ExternalInput").ap()
    wq_d = nc.dram_tensor("wqt", [D_MODEL, C], BF16, kind="ExternalInput").ap()
    wk_d = nc.dram_tensor("wkt", [D_MODEL, C], BF16, kind="ExternalInput").ap()
    wv_d = nc.dram_tensor("wvt", [D_MODEL, C], BF16, kind="ExternalInput").ap()
    wo_d = nc.dram_tensor("wot", [C, D_MODEL], BF16, kind="ExternalInput").ap()
    cos_d = nc.dram_tensor("cosb", [128, L], BF16, kind="ExternalInput").ap()
    sin_d = nc.dram_tensor("ssin", [128, L], BF16, kind="ExternalInput").ap()
    mk_d = nc.dram_tensor("masks", [128, 128], BF16, kind="ExternalInput").ap()
    eye_d = nc.dram_tensor("eye", [128, 256], BF16, kind="ExternalInput").ap()
    out_d = nc.dram_tensor("out", [L, D_MODEL], BF16, kind="ExternalOutput").ap()

    # ---- persistent SBUF tensors
    wq = pp.tile([128, 8, C], BF16)
    wk = pp.tile([128, 8, C], BF16)
    wv = pp.tile([128, 8, C], BF16)
    wo = pp.tile([128, 2, D_MODEL], BF16)
    cs = pp.tile([128, L], BF16)
    sn = pp.tile([128, L], BF16)
    mtriT = pp.tile([128, 128], BF16)   # strict upper-tri -30000 (mask bias^T)
    eye = pp.tile([128, 256], BF16)     # identity, doubled
    xb = pp.tile([128, 8, L], BF16)     # x^T, d-tile-major
    qtb = pp.tile([128, 2, L], BF16)
    ktb = pp.tile([128, 2, L], BF16)
    # V per k-tile: head hd at [128*hd, 128*hd+64), ones at [128*hd+64, ...)
    vt = pp.tile([128, n_lt, 512], BF16)
    at = pp.tile([128, 2, L], BF16)

    # ---- input DMA: everything on the sync queue (scalar stays clear for exp);
    # later chunks' x loads are emitted inside the main loop so store DMAs
    # interleave rather than queueing behind them.
    xt_t = xt_d.rearrange("(a p) l -> p a l", p=128)
    nc.sync.dma_start(out=wq[:], in_=wq_d.rearrange("(a p) c -> p a c", p=128))
    nc.sync.dma_start(out=xb[:, 0:2, 0:PCW], in_=xt_t[:, 0:2, 0:PCW])
    nc.sync.dma_start(out=xb[:, 2:4, 0:PCW], in_=xt_t[:, 2:4, 0:PCW])
    nc.sync.dma_start(out=xb[:, 4:6, 0:PCW], in_=xt_t[:, 4:6, 0:PCW])
    nc.sync.dma_start(out=xb[:, 6:8, 0:PCW], in_=xt_t[:, 6:8, 0:PCW])
    nc.sync.dma_start(out=wk[:], in_=wk_d.rearrange("(a p) c -> p a c", p=128))
    nc.sync.dma_start(out=cs[:], in_=cos_d)
    nc.sync.dma_start(out=sn[:], in_=sin_d)
    nc.sync.dma_start(out=mtriT[:], in_=mk_d)
    nc.sync.dma_start(out=eye[:], in_=eye_d)
    nc.sync.dma_start(out=wv[:], in_=wv_d.rearrange("(a p) c -> p a c", p=128))
    nc.sync.dma_start(out=xb[:, :, PCW:2 * PCW], in_=xt_t[:, :, PCW:2 * PCW])
    nc.sync.dma_start(out=wo[:], in_=wo_d.rearrange("(a p) e -> p a e", p=128))
    # warm the PE p-state during the initial DMA window: dummy matmuls on
    # a zeroed sbuf tile (results discarded; psum reset by the first real
    # start=True groups).  wrm memset first so warmups start immediately.
    wrm = pp.tile([128, 256], BF16, name="wrm")
    nc.gpsimd.memset(wrm[:], 0.0)
    for wi in range(8):
        wps = pjp.tile([128, PCW], F32, tag="pj", name=f"wps{wi}")
        nc.tensor.matmul(wps[:, 0:C], lhsT=wrm[:, 0:128], rhs=wrm[:, :],
                         start=True, stop=True, skip_group_check=True)
    # ones columns of vt (once)
    vtv = vt[:, :, :].rearrange("p k (hd x) -> p k hd x", x=128)
    nc.gpsimd.memset(vtv[:, :, :, 64:128], 1.0)

    # ---------------- streams ----------------
    def proj_qk_stream(c):
        ls = c * PCW
        if c + 2 < n_pc:  # prefetch chunk c+2 (c+1 already in flight)
            ns_ = (c + 2) * PCW
            nc.sync.dma_start(out=xb[:, :, ns_:ns_ + PCW],
                              in_=xt_t[:, :, ns_:ns_ + PCW])
        for nm, wt, dstb in (("q", wq, qtb), ("k", wk, ktb)):
            for ct in (0, 1):
                pj = pjp.tile([128, PCW], F32, tag="pj", name=f"pj_{nm}{ct}_{c}")
                for dt_ in range(8):
                    nc.tensor.matmul(
                        pj[:],
                        lhsT=wt[:, dt_, ct * 128:ct * 128 + 128],
                        rhs=xb[:, dt_, ls:ls + PCW],
                        start=(dt_ == 0), stop=(dt_ == 7))
                    if dt_ == 3:
                        yield
                # RoPE: r = pj*cos + shuffle(pj)*ssin
                sh = rpp.tile([128, PCW], F32, tag="sh", name=f"sh{nm}{ct}{c}")
                t1 = rpp.tile([128, PCW], BF16, tag="t1", name=f"t1{nm}{ct}{c}")
                t2 = rpp.tile([128, PCW], BF16, tag="t2", name=f"t2{nm}{ct}{c}")
                nc.vector.stream_shuffle(sh[:], pj[:], SWAP32)
                nc.vector.tensor_mul(t1[:], pj[:], cs[:, ls:ls + PCW])
                nc.gpsimd.tensor_mul(t2[:], sh[:], sn[:, ls:ls + PCW])
                nc.gpsimd.tensor_add(dstb[:, ct, ls:ls + PCW], t1[:], t2[:])
                yield
    def proj_v_stream(c):
        for lt in range(c * 4, c * 4 + 4):
            pj = pjp.tile([128, PCW], F32, tag="pj", name=f"pjv_{lt}")
            for dt_ in range(8):
                nc.tensor.matmul(
                    pj[:, 0:C],
                    lhsT=xb[:, dt_, lt * 128:lt * 128 + 128],
                    rhs=wv[:, dt_, :],
                    start=(dt_ == 0), stop=(dt_ == 7))
                if dt_ == 3:
                    yield
            nc.vector.tensor_copy(
                vtv[:, lt, :, 0:64],
                pj[:, 0:C].rearrange("p (hd x) -> p hd x", x=64))
            yield

    def proj_stream(c):
        yield from proj_qk_stream(c)
        yield from proj_v_stream(c)

    def attn_stream(qcs):
        for qc in qcs:
            for pair in (0, 1):
                qs = qc * PCW
                nkt = 4 * (qc + 1)
                po = pop.tile([128, 2, PCW], F32, tag="po", name=f"po_{qc}_{pair}")
                tiles = {}

                def qk(kt):
                    pt_ps = scp.tile([128, 2, PCW], F32, tag="sc",
                                     name=f"ps_{qc}_{pair}_{kt}")
                    pt_sb = sbp.tile([128, 2, PCW], BF16, tag="pt",
                                     name=f"pb_{qc}_{pair}_{kt}")
                    qlo = max(0, kt * 128 - qs)
                    diag = kt * 128 >= qs
                    for h in (0, 1):
                        nc.tensor.matmul(
                            pt_ps[:, h, qlo:PCW],
                            lhsT=ktb[64 * h:64 * h + 64, pair,
                                     kt * 128:kt * 128 + 128],
                            rhs=qtb[64 * h:64 * h + 64, pair,
                                    qs + qlo:qs + PCW],
                            start=True, stop=not diag,
                            tile_position=(64 * h, 0),
                            skip_group_check=True)
                    if diag:
                        boff = kt * 128 - qs
                        nc.tensor.matmul(
                            pt_ps[:, :, boff:boff + 128],
                            lhsT=mtriT[:, 0:128],
                            rhs=eye[:, :],
                            start=False, stop=True,
                            skip_group_check=True)
                    tiles[kt] = (pt_ps, pt_sb, qlo)

                def exp_pv(kt):
                    pt_ps, pt_sb, qlo = tiles.pop(kt)
                    nc.scalar.activation(
                        pt_sb[:, :, qlo:PCW], pt_ps[:, :, qlo:PCW],
                        mybir.ActivationFunctionType.Exp, scale=SCALE)
                    for h in (0, 1):
                        hd = 2 * pair + h
                        nc.tensor.matmul(
                            po[:, h, qlo:PCW],
                            lhsT=vt[:, kt, 128 * hd:128 * hd + 128],
                            rhs=pt_sb[:, h, qlo:PCW],
                            start=(kt == 0),
                            stop=(kt == nkt - 1),
                            skip_group_check=True)

                for kt in range(nkt):
                    qk(kt)
                    yield
                    if kt > 0:
                        exp_pv(kt - 1)
                        yield
                exp_pv(nkt - 1)
                # drain: denominators live in po[64:128] (ones-replicated).
                # Standard tensor_copy honors a shifted input base partition;
                # custom-DVE ops (reciprocal) do not -- copy down first.
                dcp = drp.tile([64, 2, PCW], F32, tag="dcp", name=f"dc_{qc}_{pair}")
                nc.vector.tensor_copy(dcp[:], po[64:128, :, :])
                pbi = drp.tile([64, 2, PCW], F32, tag="pbi", name=f"pi_{qc}_{pair}")
                nc.vector.reciprocal_approx_fast(out=pbi[:], in_=dcp[:])
                tmh = drp.tile([64, PCW], BF16, tag="tmh", name=f"th_{qc}_{pair}")
                nc.vector.tensor_mul(tmh[:], po[0:64, 1, :], pbi[:, 1, :])
                nc.sync.dma_start(out=at[64:128, pair, qs:qs + PCW], in_=tmh[:])
                nc.vector.tensor_mul(at[0:64, pair, qs:qs + PCW],
                                     po[0:64, 0, :], pbi[:, 0, :])
                yield

    def oproj_stream(lts, split_q=False):
        for lt in lts:
            for eh in (0, 1):
                pj = pjp.tile([128, PCW], F32, tag="pj", name=f"pjo_{lt}_{eh}")
                nc.tensor.matmul(pj[:], lhsT=at[:, 0, lt * 128:lt * 128 + 128],
                                 rhs=wo[:, 0, eh * 512:eh * 512 + 512],
                                 start=True, stop=False, skip_group_check=True)
                yield
                nc.tensor.matmul(pj[:], lhsT=at[:, 1, lt * 128:lt * 128 + 128],
                                 rhs=wo[:, 1, eh * 512:eh * 512 + 512],
                                 start=False, stop=True, skip_group_check=True)
                yield
                stg = osp.tile([128, PCW], BF16, tag="stg", name=f"stg_{lt}_{eh}")
                nc.vector.tensor_copy(stg[:], pj[:])
                seng = nc.scalar if (split_q and eh == 1) else nc.sync
                seng.dma_start(out=out_d[lt * 128:lt * 128 + 128,
                                         eh * 512:eh * 512 + 512],
                               in_=stg[:])
                yield

    def run_all(gen):
        for _ in gen:
            pass

    def interleave(a, b, na, nb):
        """Pace nb filler units (b) evenly across na units of a."""
        ita, itb = iter(a), iter(b)
        alive_a = alive_b = True
        acc = 0.0
        step = nb / max(1, na)
        while alive_a or alive_b:
            if alive_a:
                try:
                    next(ita)
                except StopIteration:
                    alive_a = False
            acc += step if alive_a else 1e9
            while alive_b and acc >= 1.0:
                acc -= 1.0
                try:
                    next(itb)
                except StopIteration:
                    alive_b = False

    def chain(*gens):
        for g in gens:
            yield from g

    def n_attn(qcs):
        return sum(2 * (2 * 4 * (qc + 1) + 1) for qc in qcs)

    N_PROJ = 16  # yields per proj_stream chunk
    N_OP = 6     # yields per oproj l-tile

    run_all(proj_stream(0))
    interleave(attn_stream((0,)), proj_stream(1), n_attn((0,)), N_PROJ)
    interleave(attn_stream((1,)),
               chain(proj_stream(2), oproj_stream((0, 1, 2))),
               n_attn((1,)), N_PROJ + 3 * N_OP)
    interleave(attn_stream((2,)),
               chain(proj_qk_stream(3), oproj_stream((3, 4, 5))),
               n_attn((2,)), 8 + 3 * N_OP)
    interleave(attn_stream((3,)),
               chain(proj_v_stream(3), oproj_stream((6, 7, 8, 9, 10, 11))),
               n_attn((3,)), 8 + 6 * N_OP)
    run_all(oproj_stream((12, 13, 14, 15), split_q=True))


def build_nc(L=2048):
    """Build + compile the per-core Bass program (same NEFF on all 8 cores)."""
    assert L % PCW == 0
    nc = bacc.Bacc("TRN2", target_bir_lowering=False, debug=False,
                   num_devices=N_CORES)
    with tile.TileContext(nc) as tc:
        with tc.tile_pool(name="persist", bufs=1) as pp, \
             tc.tile_pool(name="ropet", bufs=2) as rpp, \
             tc.tile_pool(name="ptsb", bufs=4) as sbp, \
             tc.tile_pool(name="drain", bufs=4) as drp, \
             tc.tile_pool(name="ostg", bufs=3) as osp, \
             tc.tile_pool(name="sc_ps", bufs=2, space="PSUM") as scp, \
             tc.tile_pool(name="po_ps", bufs=1, space="PSUM") as pop, \
             tc.tile_pool(name="pj_ps", bufs=2, space="PSUM") as pjp:
            _body(nc, tc, L, pp, rpp, sbp, drp, osp, scp, pop, pjp)
    nc.compile()
    return nc


_NC_CACHE = {}


def _get_nc(L):
    if L not in _NC_CACHE:
        _NC_CACHE[L] = build_nc(L)
    return _NC_CACHE[L]


def make_inputs(x, token_positions, Wq, Wk, Wv, Wo):
    """Host-side shard/layout prep -> list of 8 per-core input dicts."""
    B, L, _ = x.shape
    pos = np.asarray(token_positions).astype(np.float64)
    S = ROPE_THETA ** (-2.0 / D_HEAD)
    thetas = S ** np.arange(HALF, dtype=np.float64)
    ang = pos[:, None] * thetas[None, :]          # [L, 32]
    cosL = np.cos(ang).T                          # [32, L]
    sinL = np.sin(ang).T
    # per-channel tables on the natural (head, dim) layout:
    # row p (within a 64-row head block): pair i = (p%64)//2
    # cosb[p] = cos(theta_i * pos); ssin[p] = -sin if dim even else +sin
    cosb = np.empty((128, L), dtype=np.float64)
    ssin = np.empty((128, L), dtype=np.float64)
    for p in range(128):
        i = (p % 64) // 2
        cosb[p] = cosL[i]
        ssin[p] = -sinL[i] if (p % 2 == 0) else sinL[i]
    cosb = cosb.astype(NPBF16)
    ssin = ssin.astype(NPBF16)

    r = np.arange(128)[:, None]
    col = np.arange(128)[None, :]
    # masks = mtriT: strict upper-triangular -30000.  Used as matmul lhsT
    # with rhs=I to add -30000 above the diagonal of score blocks (so the
    # device adds bias[p, j] = mtriT[j, p]... lhsT[d, p] applied as
    # (lhsT.T @ I)[p, j] = mtriT[j, p]; want -30000 where j < p.
    masks = np.where(r < col, -30000.0, 0.0).astype(NPBF16)
    eye = np.concatenate([np.eye(128), np.eye(128)], axis=1).astype(NPBF16)

    xts = [np.ascontiguousarray(x[b].astype(NPBF16).T) for b in range(B)]
    in_maps = []
    shard_cache = {}
    for core in range(N_CORES):
        b, hg = core // 4, core % 4
        if hg not in shard_cache:
            rows = slice(hg * 256, hg * 256 + 256)
            shard_cache[hg] = {
                "wqt": np.ascontiguousarray(Wq[rows].astype(NPBF16).T),
                "wkt": np.ascontiguousarray(Wk[rows].astype(NPBF16).T),
                "wvt": np.ascontiguousarray(Wv[rows].astype(NPBF16).T),
                "wot": np.ascontiguousarray(Wo[:, rows].astype(NPBF16).T),
            }
        m = dict(shard_cache[hg])
        m["xt"] = xts[b]
        m["cosb"] = cosb
        m["ssin"] = ssin
        m["masks"] = masks
        m["eye"] = eye
        in_maps.append(m)
    return in_maps


def kernel(x, token_positions, Wq, Wk, Wv, Wo):
    x = np.asarray(x); Wq = np.asarray(Wq); Wk = np.asarray(Wk)
    Wv = np.asarray(Wv); Wo = np.asarray(Wo)
    B, L, _ = x.shape
    nc = _get_nc(L)
    in_maps = make_inputs(x, token_positions, Wq, Wk, Wv, Wo)
    res = run_bass_kernel_spmd(nc, in_maps, core_ids=list(range(N_CORES)))
    out = np.zeros((B, L, D_MODEL), dtype=np.float32)
    for core in range(N_CORES):
        out[core // 4] += res.results[core]["out"].astype(np.float32)
    return out


# revision 41
# speedup vs baseline: 1.0128x; 1.0061x over previous
"""Multi-head self-attention (RoPE, causal) Trainium2 Bass kernel, 8 NeuronCores.

Sharding: data-parallel over batch (B=2) x tensor-parallel over heads
(16 heads -> 4 groups of 4). Core c handles batch b=c//4, heads 4*(c%4)..4*(c%4)+3.
Each core computes its 4 heads' attention plus a partial output projection;
the host sums the 4 bf16 partial outputs per batch element in f32.

Single software-pipelined phase per core (vs the old serial phases):
QKV projection of chunk c+1, attention for chunk c, and output
projection (per l-tile, gated on its q-chunk's drain) are EMITTED
interleaved at matmul granularity with fractional pacing, so the
in-order tensor-engine queue always has filler work during the
activation engine's softmax-exp latency (exp is the co-bottleneck:
~80us at 128 lanes/cycle incl. overheads, vs ~100us of PE streaming).

Layouts / key tricks (per core):
  x^T [1024d, L] bf16 as one [128, 8, L] tile, DMA'd in 512-col chunks,
  pipelined 2 chunks ahead; 8 dummy warm-up matmuls ramp the PE p-state
  during the initial DMA window.
  Q^T/K^T [256c, L] = W_slice @ x^T; RoPE in [channel, L] layout via
  DVE stream_shuffle pair-swap + cos/signed-sin (shuffle+mul DVE,
  mul+add GpSimd; GpSimd cannot read PSUM).
  Scores T = K^T.T @ Q^T per head in [k, q] 512-wide q-chunks; the two
  heads' QK matmuls stream CONCURRENTLY via tile_position (0,0)/(64,0).
  Causal masking costs no vector work: a tiny extra matmul accumulates
  -30000 above the diagonal in PSUM (lhsT = strict-upper-tri const,
  rhs = doubled identity covering both heads), so exp -> 0.
  exp per k-tile covers both heads [128, <=1024] on the scalar engine.
  PV: P^T @ [V_h | ones*64] stationary M=128 -- the ones block
  replicated 64x makes po[64:128] hold the softmax denominators
  pre-broadcast across 64 partitions: drain = shifted tensor_copy down
  (standard copy honors shifted base partitions; custom-DVE ops do
  not), reciprocal_approx_fast, two muls; no DRAM round-trip.
  PSUM discipline: first writer of each 2KB bank uses start=True
  (marks the whole zero-region), later disjoint writers start=False
  accumulate onto pending-zero.  8 banks: scores 2x2, po 2, proj 2.
  Output projection per l-tile from at [256c, L] x Wo^T, staged bf16
  on DVE, bf16 partials summed on host in f32.
"""
import sys, math

sys.path.insert(0, "/opt/trn_rl_repo")

import numpy as np
import ml_dtypes

import concourse.bacc as bacc
import concourse.bass as bass
import concourse.mybir as mybir
import concourse.tile as tile
from concourse.bass_utils import run_bass_kernel_spmd

BF16 = mybir.dt.bfloat16
F32 = mybir.dt.float32
NPBF16 = ml_dtypes.bfloat16

D_MODEL = 1024
D_HEAD = 64
HALF = D_HEAD // 2
ROPE_THETA = 10000.0
N_CORES = 8
C = 256            # channels per core (4 heads x 64)
PCW = 512          # projection chunk width (L columns)
QW = 256           # attention q-chunk width
SWAP32 = [i ^ 1 for i in range(32)]
SCALE = 1.0 / math.sqrt(D_HEAD)


def _body(nc, tc, L, pp, rpp, sbp, drp, osp, scp, pop, pjp):
    n_pc = L // PCW          # projection chunks
    n_qc = L // QW           # attention q chunks
    n_lt = L // 128          # l-tiles / k-tiles

    xt_d = nc.dram_tensor("xt", [D_MODEL, L], BF16, kind="ExternalInput").ap()
    wq_d = nc.dram_tensor("wqt", [D_MODEL, C], BF16, kind="ExternalInput").ap()
    wk_d = nc.dram_tensor("wkt", [D_MODEL, C], BF16, kind="ExternalInput").ap()
    wv_d = nc.dram_tensor("wvt", [D_MODEL, C], BF16, kind="ExternalInput").ap()
    wo_d = nc.dram_tensor("wot", [C, D_MODEL], BF16, kind="ExternalInput").ap()
    cos_d = nc.dram_tensor("cosb", [128, L], BF16, kind="ExternalInput").ap()
    sin_d = nc.dram_tensor("ssin", [128, L], BF16, kind="ExternalInput").ap()
    mk_d = nc.dram_tensor("masks", [128, 128], BF16, kind="ExternalInput").ap()
    eye_d = nc.dram_tensor("eye", [128, 256], BF16, kind="ExternalInput").ap()
    out_d = nc.dram_tensor("out", [L, D_MODEL], BF16, kind="ExternalOutput").ap()

    # ---- persistent SBUF tensors
    wq = pp.tile([128, 8, C], BF16)
    wk = pp.tile([128, 8, C], BF16)
    wv = pp.tile([128, 8, C], BF16)
    wo = pp.tile([128, 2, D_MODEL], BF16)
    cs = pp.tile([128, L], BF16)
    sn = pp.tile([128, L], BF16)
    mtriT = pp.tile([128, 128], BF16)   # strict upper-tri -30000 (mask bias^T)
    eye = pp.tile([128, 256], BF16)     # identity, doubled
    xb = pp.tile([128, 8, L], BF16)     # x^T, d-tile-major
    qtb = pp.tile([128, 2, L], BF16)
    ktb = pp.tile([128, 2, L], BF16)
    # V per k-tile: head hd at [128*hd, 128*hd+64), ones at [128*hd+64, ...)
    vt = pp.tile([128, n_lt, 512], BF16)
    at = pp.tile([128, 2, L], BF16)

    # ---- input DMA: everything on the sync queue (scalar stays clear for exp);
    # later chunks' x loads are emitted inside the main loop so store DMAs
    # interleave rather than queueing behind them.
    xt_t = xt_d.rearrange("(a p) l -> p a l", p=128)
    wq_r = wq_d.rearrange("(a p) c -> p a c", p=128)
    nc.sync.dma_start(out=wq[:, :, 0:128], in_=wq_r[:, :, 0:128])
    nc.sync.dma_start(out=xb[:, 0:2, 0:PCW], in_=xt_t[:, 0:2, 0:PCW])
    nc.sync.dma_start(out=xb[:, 2:4, 0:PCW], in_=xt_t[:, 2:4, 0:PCW])
    nc.sync.dma_start(out=xb[:, 4:6, 0:PCW], in_=xt_t[:, 4:6, 0:PCW])
    nc.sync.dma_start(out=xb[:, 6:8, 0:PCW], in_=xt_t[:, 6:8, 0:PCW])
    nc.sync.dma_start(out=wq[:, :, 128:256], in_=wq_r[:, :, 128:256])
    nc.sync.dma_start(out=wk[:], in_=wk_d.rearrange("(a p) c -> p a c", p=128))
    nc.sync.dma_start(out=cs[:], in_=cos_d)
    nc.sync.dma_start(out=sn[:], in_=sin_d)
    nc.sync.dma_start(out=mtriT[:], in_=mk_d)
    nc.sync.dma_start(out=eye[:], in_=eye_d)
    nc.sync.dma_start(out=wv[:], in_=wv_d.rearrange("(a p) c -> p a c", p=128))
    nc.sync.dma_start(out=xb[:, :, PCW:2 * PCW], in_=xt_t[:, :, PCW:2 * PCW])
    nc.sync.dma_start(out=wo[:], in_=wo_d.rearrange("(a p) e -> p a e", p=128))
    # warm the PE p-state during the initial DMA window: dummy matmuls on
    # a zeroed sbuf tile (results discarded; psum reset by the first real
    # start=True groups).  wrm memset first so warmups start immediately.
    wrm = pp.tile([128, 256], BF16, name="wrm")
    nc.gpsimd.memset(wrm[:], 0.0)
    for wi in range(8):
        wps = pjp.tile([128, PCW], F32, tag="pj", name=f"wps{wi}")
        nc.tensor.matmul(wps[:, 0:C], lhsT=wrm[:, 0:128], rhs=wrm[:, :],
                         start=True, stop=True, skip_group_check=True)
    # ones columns of vt (once)
    vtv = vt[:, :, :].rearrange("p k (hd x) -> p k hd x", x=128)
    nc.gpsimd.memset(vtv[:, :, :, 64:128], 1.0)

    # ---------------- streams ----------------
    def proj_qk_stream(c):
        ls = c * PCW
        if c + 2 < n_pc:  # prefetch chunk c+2 (c+1 already in flight)
            ns_ = (c + 2) * PCW
            nc.sync.dma_start(out=xb[:, :, ns_:ns_ + PCW],
                              in_=xt_t[:, :, ns_:ns_ + PCW])
        for nm, wt, dstb in (("q", wq, qtb), ("k", wk, ktb)):
            for ct in (0, 1):
                pj = pjp.tile([128, PCW], F32, tag="pj", name=f"pj_{nm}{ct}_{c}")
                for dt_ in range(8):
                    nc.tensor.matmul(
                        pj[:],
                        lhsT=wt[:, dt_, ct * 128:ct * 128 + 128],
                        rhs=xb[:, dt_, ls:ls + PCW],
                        start=(dt_ == 0), stop=(dt_ == 7))
                    if dt_ == 3:
                        yield
                # RoPE: r = pj*cos + shuffle(pj)*ssin
                sh = rpp.tile([128, PCW], F32, tag="sh", name=f"sh{nm}{ct}{c}")
                t1 = rpp.tile([128, PCW], BF16, tag="t1", name=f"t1{nm}{ct}{c}")
                t2 = rpp.tile([128, PCW], BF16, tag="t2", name=f"t2{nm}{ct}{c}")
                nc.vector.stream_shuffle(sh[:], pj[:], SWAP32)
                nc.vector.tensor_mul(t1[:], pj[:], cs[:, ls:ls + PCW])
                nc.gpsimd.tensor_mul(t2[:], sh[:], sn[:, ls:ls + PCW])
                nc.gpsimd.tensor_add(dstb[:, ct, ls:ls + PCW], t1[:], t2[:])
                yield
    def proj_v_stream(c):
        for lt in range(c * 4, c * 4 + 4):
            pj = pjp.tile([128, PCW], F32, tag="pj", name=f"pjv_{lt}")
            for dt_ in range(8):
                nc.tensor.matmul(
                    pj[:, 0:C],
                    lhsT=xb[:, dt_, lt * 128:lt * 128 + 128],
                    rhs=wv[:, dt_, :],
                    start=(dt_ == 0), stop=(dt_ == 7))
                if dt_ == 3:
                    yield
            nc.vector.tensor_copy(
                vtv[:, lt, :, 0:64],
                pj[:, 0:C].rearrange("p (hd x) -> p hd x", x=64))
            yield

    def proj_stream(c):
        yield from proj_qk_stream(c)
        yield from proj_v_stream(c)

    def attn_stream(qcs):
        for qc in qcs:
            for pair in (0, 1):
                qs = qc * PCW
                nkt = 4 * (qc + 1)
                po = pop.tile([128, 2, PCW], F32, tag="po", name=f"po_{qc}_{pair}")
                tiles = {}

                def qk(kt):
                    pt_ps = scp.tile([128, 2, PCW], F32, tag="sc",
                                     name=f"ps_{qc}_{pair}_{kt}")
                    pt_sb = sbp.tile([128, 2, PCW], BF16, tag="pt",
                                     name=f"pb_{qc}_{pair}_{kt}")
                    qlo = max(0, kt * 128 - qs)
                    diag = kt * 128 >= qs
                    for h in (0, 1):
                        nc.tensor.matmul(
                            pt_ps[:, h, qlo:PCW],
                            lhsT=ktb[64 * h:64 * h + 64, pair,
                                     kt * 128:kt * 128 + 128],
                            rhs=qtb[64 * h:64 * h + 64, pair,
                                    qs + qlo:qs + PCW],
                            start=True, stop=not diag,
                            tile_position=(64 * h, 0),
                            skip_group_check=True)
                    if diag:
                        boff = kt * 128 - qs
                        nc.tensor.matmul(
                            pt_ps[:, :, boff:boff + 128],
                            lhsT=mtriT[:, 0:128],
                            rhs=eye[:, :],
                            start=False, stop=True,
                            skip_group_check=True)
                    tiles[kt] = (pt_ps, pt_sb, qlo)

                def exp_pv(kt):
                    pt_ps, pt_sb, qlo = tiles.pop(kt)
                    nc.scalar.activation(
                        pt_sb[:, :, qlo:PCW], pt_ps[:, :, qlo:PCW],
                        mybir.ActivationFunctionType.Exp, scale=SCALE)
                    for h in (0, 1):
                        hd = 2 * pair + h
                        nc.tensor.matmul(
                            po[:, h, qlo:PCW],
                            lhsT=vt[:, kt, 128 * hd:128 * hd + 128],
                            rhs=pt_sb[:, h, qlo:PCW],
                            start=(kt == 0),
                            stop=(kt == nkt - 1),
                            skip_group_check=True)

                for kt in range(nkt):
                    qk(kt)
                    yield
                    if kt > 0:
                        exp_pv(kt - 1)
                        yield
                exp_pv(nkt - 1)
                # drain: denominators live in po[64:128] (ones-replicated).
                # Standard tensor_copy honors a shifted input base partition;
                # custom-DVE ops (reciprocal) do not -- copy down first.
                dcp = drp.tile([64, 2, PCW], F32, tag="dcp", name=f"dc_{qc}_{pair}")
                nc.vector.tensor_copy(dcp[:], po[64:128, :, :])
                pbi = drp.tile([64, 2, PCW], F32, tag="pbi", name=f"pi_{qc}_{pair}")
                nc.vector.reciprocal_approx_fast(out=pbi[:], in_=dcp[:])
                tmh = drp.tile([64, PCW], BF16, tag="tmh", name=f"th_{qc}_{pair}")
                nc.vector.tensor_mul(tmh[:], po[0:64, 1, :], pbi[:, 1, :])
                nc.sync.dma_start(out=at[64:128, pair, qs:qs + PCW], in_=tmh[:])
                nc.vector.tensor_mul(at[0:64, pair, qs:qs + PCW],
                                     po[0:64, 0, :], pbi[:, 0, :])
                yield

    def oproj_stream(lts, split_q=False):
        for lt in lts:
            for eh in (0, 1):
                pj = pjp.tile([128, PCW], F32, tag="pj", name=f"pjo_{lt}_{eh}")
                nc.tensor.matmul(pj[:], lhsT=at[:, 0, lt * 128:lt * 128 + 128],
                                 rhs=wo[:, 0, eh * 512:eh * 512 + 512],
                                 start=True, stop=False, skip_group_check=True)
                yield
                nc.tensor.matmul(pj[:], lhsT=at[:, 1, lt * 128:lt * 128 + 128],
                                 rhs=wo[:, 1, eh * 512:eh * 512 + 512],
                                 start=False, stop=True, skip_group_check=True)
                yield
                stg = osp.tile([128, PCW], BF16, tag="stg", name=f"stg_{lt}_{eh}")
                nc.vector.tensor_copy(stg[:], pj[:])
                seng = nc.scalar if (split_q and eh == 1) else nc.sync
                seng.dma_start(out=out_d[lt * 128:lt * 128 + 128,
                                         eh * 512:eh * 512 + 512],
                               in_=stg[:])
                yield

    def run_all(gen):
        for _ in gen:
            pass

    def interleave(a, b, na, nb):
        """Pace nb filler units (b) evenly across na units of a."""
        ita, itb = iter(a), iter(b)
        alive_a = alive_b = True
        acc = 0.0
        step = nb / max(1, na)
        while alive_a or alive_b:
            if alive_a:
                try:
                    next(ita)
                except StopIteration:
                    alive_a = False
            acc += step if alive_a else 1e9
            while alive_b and acc >= 1.0:
                acc -= 1.0
                try:
                    next(itb)
                except StopIteration:
                    alive_b = False

    def chain(*gens):
        for g in gens:
            yield from g

    def n_attn(qcs):
        return sum(2 * (2 * 4 * (qc + 1) + 1) for qc in qcs)

    N_PROJ = 16  # yields per proj_stream chunk
    N_OP = 6     # yields per oproj l-tile

    run_all(proj_stream(0))
    interleave(attn_stream((0,)), proj_stream(1), n_attn((0,)), N_PROJ)
    interleave(attn_stream((1,)),
               chain(proj_stream(2), oproj_stream((0, 1, 2))),
               n_attn((1,)), N_PROJ + 3 * N_OP)
    interleave(attn_stream((2,)),
               chain(proj_qk_stream(3), oproj_stream((3, 4, 5))),
               n_attn((2,)), 8 + 3 * N_OP)
    interleave(attn_stream((3,)),
               chain(proj_v_stream(3), oproj_stream((6, 7, 8, 9, 10, 11))),
               n_attn((3,)), 8 + 6 * N_OP)
    run_all(oproj_stream((12, 13, 14, 15), split_q=True))


def build_nc(L=2048):
    """Build + compile the per-core Bass program (same NEFF on all 8 cores)."""
    assert L % PCW == 0
    nc = bacc.Bacc("TRN2", target_bir_lowering=False, debug=False,
                   num_devices=N_CORES)
    with tile.TileContext(nc) as tc:
        with tc.tile_pool(name="persist", bufs=1) as pp, \
             tc.tile_pool(name="ropet", bufs=2) as rpp, \
             tc.tile_pool(name="ptsb", bufs=4) as sbp, \
             tc.tile_pool(name="drain", bufs=4) as drp, \
             tc.tile_pool(name="ostg", bufs=3) as osp, \
             tc.tile_pool(name="sc_ps", bufs=2, space="PSUM") as scp, \
             tc.tile_pool(name="po_ps", bufs=1, space="PSUM") as pop, \
             tc.tile_pool(name="pj_ps", bufs=2, space="PSUM") as pjp:
            _body(nc, tc, L, pp, rpp, sbp, drp, osp, scp, pop, pjp)
    nc.compile()
    return nc


_NC_CACHE = {}


def _get_nc(L):
    if L not in _NC_CACHE:
        _NC_CACHE[L] = build_nc(L)
    return _NC_CACHE[L]


def make_inputs(x, token_positions, Wq, Wk, Wv, Wo):
    """Host-side shard/layout prep -> list of 8 per-core input dicts."""
    B, L, _ = x.shape
    pos = np.asarray(token_positions).astype(np.float64)
    S = ROPE_THETA ** (-2.0 / D_HEAD)
    thetas = S ** np.arange(HALF, dtype=np.float64)
    ang = pos[:, None] * thetas[None, :]          # [L, 32]
    cosL = np.cos(ang).T                          # [32, L]
    sinL = np.sin(ang).T
    # per-channel tables on the natural (head, dim) layout:
    # row p (within a 64-row head block): pair i = (p%64)//2
    # cosb[p] = cos(theta_i * pos); ssin[p] = -sin if dim even else +sin
    cosb = np.empty((128, L), dtype=np.float64)
    ssin = np.empty((128, L), dtype=np.float64)
    for p in range(128):
        i = (p % 64) // 2
        cosb[p] = cosL[i]
        ssin[p] = -sinL[i] if (p % 2 == 0) else sinL[i]
    cosb = cosb.astype(NPBF16)
    ssin = ssin.astype(NPBF16)

    r = np.arange(128)[:, None]
    col = np.arange(128)[None, :]
    # masks = mtriT: strict upper-triangular -30000.  Used as matmul lhsT
    # with rhs=I to add -30000 above the diagonal of score blocks (so the
    # device adds bias[p, j] = mtriT[j, p]... lhsT[d, p] applied as
    # (lhsT.T @ I)[p, j] = mtriT[j, p]; want -30000 where j < p.
    masks = np.where(r < col, -30000.0, 0.0).astype(NPBF16)
    eye = np.concatenate([np.eye(128), np.eye(128)], axis=1).astype(NPBF16)

    xts = [np.ascontiguousarray(x[b].astype(NPBF16).T) for b in range(B)]
    in_maps = []
    shard_cache = {}
    for core in range(N_CORES):
        b, hg = core // 4, core % 4
        if hg not in shard_cache:
            rows = slice(hg * 256, hg * 256 + 256)
            shard_cache[hg] = {
                "wqt": np.ascontiguousarray(Wq[rows].astype(NPBF16).T),
                "wkt": np.ascontiguousarray(Wk[rows].astype(NPBF16).T),
                "wvt": np.ascontiguousarray(Wv[rows].astype(NPBF16).T),
                "wot": np.ascontiguousarray(Wo[:, rows].astype(NPBF16).T),
            }
        m = dict(shard_cache[hg])
        m["xt"] = xts[b]
        m["cosb"] = cosb
        m["ssin"] = ssin
        m["masks"] = masks
        m["eye"] = eye
        in_maps.append(m)
    return in_maps


def kernel(x, token_positions, Wq, Wk, Wv, Wo):
    x = np.asarray(x); Wq = np.asarray(Wq); Wk = np.asarray(Wk)
    Wv = np.asarray(Wv); Wo = np.asarray(Wo)
    B, L, _ = x.shape
    nc = _get_nc(L)
    in_maps = make_inputs(x, token_positions, Wq, Wk, Wv, Wo)
    res = run_bass_kernel_spmd(nc, in_maps, core_ids=list(range(N_CORES)))
    out = np.zeros((B, L, D_MODEL), dtype=np.float32)
    for core in range(N_CORES):
        out[core // 4] += res.results[core]["out"].astype(np.float32)
    return out
